# revision 12
# baseline (speedup 1.0000x reference)
"""3-layer GAT on 8 Trainium2 NeuronCores (Bass/Tile).

Strategy (edge-parallel, dst-sharded):
  - dst nodes are split into 8 contiguous slices (6250/core); each core owns all
    edges pointing into its slice (edges sorted by dst on host).
  - Node table rows are 384 bf16 elems (768B): [h (256) | al_src (4) | pad].
  - Per layer: sharded dense projection H_k = X_k @ [W | w_als | w_ald] on PE
    (the extra columns produce the a_src/a_dst dot products for free), an
    AllGather of the augmented bf16 node table, then the edge phase:
    dma_gather of each edge's source row (<=896 idx per gather: the SWDGE
    descriptor ring holds ~1K descriptors and the ANT gather does not pace),
    attention logits via a per-chunk one-hot "expansion" matmul
    (al_dst -> per-edge), exp without max-subtraction (logits verified < 9),
    and segment softmax + weighted message sum via matmuls against
    host-precomputed one-hot S matrices (fp8).
  - Layer 1 needs no collective: x is a replicated input, so each core
    computes the full projected table locally.
  - Graph mean-pool is a matmul against a one-hot pooling matrix; final
    linear on PE; the host sums the 8 partial outputs (graphs straddle core
    boundaries) and adds the output bias.
"""
import sys
import os

for _p in ('/opt/trn_rl_repo',):
    if _p not in sys.path:
        sys.path.insert(0, _p)

import numpy as np
import ml_dtypes

BF16 = ml_dtypes.bfloat16
FP8 = ml_dtypes.float8_e4m3

LAST_HW_NS = 0
ROW = 384             # table row elems (bf16) = 768B
MAX_GATHER_SLOTS = 7  # <=896 idxs per dma_gather (SWDGE ring capacity ~1K descs)


class Cfg:
    def __init__(self, N=50000, E=800000, F_IN=128, H=4, C=64, NGRAPH=512,
                 NCOUT=10, NCORES=8, SPLIT_ID=25600, TILE_BATCH=3, neg=0.2):
        self.N, self.E, self.F_IN, self.H, self.C = N, E, F_IN, H, C
        self.HC = H * C
        self.NGRAPH, self.NCOUT, self.NCORES = NGRAPH, NCOUT, NCORES
        self.neg = neg
        assert N % NCORES == 0
        self.NPC = N // NCORES                      # nodes per core
        self.NT = (self.NPC + 127) // 128           # dst tiles per core
        self.TLAST = self.NPC - 128 * (self.NT - 1)
        self.NODE_PAD = self.NT * 128
        self.N_ALL_PAD = ((N + 127) // 128) * 128   # layer-1 table rows
        self.SPLIT_ID = min(SPLIT_ID, N)            # src group boundary (node id)
        assert self.SPLIT_ID <= 32767
        assert self.N_ALL_PAD - self.SPLIT_ID <= 32768
        self.TILE_BATCH = TILE_BATCH
        self.NBATCH = (self.NT + TILE_BATCH - 1) // TILE_BATCH


def build_schedule(cfg, src, dst):
    """Sort edges by dst, slice per core, build the static (shared) chunk
    schedule and the per-core gather-index / one-hot-S arrays."""
    order = np.argsort(dst, kind='stable')
    ss, ds = src[order], dst[order]
    starts = np.searchsorted(ds, np.arange(cfg.NCORES + 1) * cfg.NPC)

    per_core = []
    counts = np.zeros((cfg.NCORES, cfg.NT, 2), np.int64)
    for k in range(cfg.NCORES):
        es = ss[starts[k]:starts[k + 1]]
        ed = ds[starts[k]:starts[k + 1]] - cfg.NPC * k
        t = ed >> 7
        g = (es >= cfg.SPLIT_ID).astype(np.int64)
        key = t * 2 + g
        o2 = np.argsort(key, kind='stable')
        es, ed, t, g, key = es[o2], ed[o2], t[o2], g[o2], key[o2]
        counts[k] = np.bincount(key, minlength=cfg.NT * 2).reshape(cfg.NT, 2)
        per_core.append((es, ed, t, g, key))

    maxc = counts.max(axis=0)                       # [NT, 2]
    slots = (maxc + 127) // 128                     # chunks per (tile, group)

    # batch-major slot layout: for b: for g: for t in batch: slots[t, g]
    slot_tile = []       # tile id of each global slot
    slot_base = np.zeros((cfg.NT, 2), np.int64)
    batches = []         # per batch: dict(g -> (first_slot, n_slots), tiles, NS)
    cur = 0
    for b in range(cfg.NBATCH):
        tb = list(range(b * cfg.TILE_BATCH, min((b + 1) * cfg.TILE_BATCH, cfg.NT)))
        ginfo = {}
        for g in (0, 1):
            first = cur
            for t in tb:
                slot_base[t, g] = cur
                for _ in range(int(slots[t, g])):
                    slot_tile.append(t)
                cur += int(slots[t, g])
            ginfo[g] = (first, cur - first)
        batches.append(dict(tiles=tb, g=ginfo, first=ginfo[0][0], NS=cur - ginfo[0][0]))
    NSLOT = cur

    # idx column offsets per (batch, g): L = 8 * n_slots
    idx_off = {}
    off = [0, 0]
    for b, binfo in enumerate(batches):
        for g in (0, 1):
            n = binfo['g'][g][1]
            idx_off[(b, g)] = off[g]
            off[g] += 8 * n
    idx_cols = off  # total columns per g

    core_data = []
    for k in range(cfg.NCORES):
        es, ed, t, g, key = per_core[k]
        ne = len(es)
        # position within (t, g) group
        grp_start_flat = np.zeros(cfg.NT * 2 + 1, np.int64)
        np.cumsum(np.bincount(key, minlength=cfg.NT * 2), out=grp_start_flat[1:])
        pos = np.arange(ne) - grp_start_flat[key]
        gslot = slot_base[t, g] + (pos >> 7)
        lane = pos & 127
        dstloc = ed - 128 * t

        idx_id = es - np.where(g == 1, cfg.SPLIT_ID, 0)
        assert idx_id.max(initial=0) <= 32767

        # wrapped idx arrays [16, cols] -> tiled to [128, cols]
        idx_a = [np.zeros((16, max(idx_cols[0], 1)), np.int16),
                 np.zeros((16, max(idx_cols[1], 1)), np.int16)]
        bidx = t // cfg.TILE_BATCH
        first_slot = np.array([[batches[b]['g'][gg][0] for gg in (0, 1)]
                               for b in range(cfg.NBATCH)], np.int64)
        q = (gslot - first_slot[bidx, g]) * 128 + lane
        colbase = np.array([[idx_off[(b, gg)] for gg in (0, 1)]
                            for b in range(cfg.NBATCH)], np.int64)
        col = colbase[bidx, g] + (q >> 4)
        row = q & 15
        for gg in (0, 1):
            m = g == gg
            idx_a[gg][row[m], col[m]] = idx_id[m].astype(np.int16)

        Sagg = np.zeros((128, max(NSLOT, 1), 128), np.float32)
        Sagg[lane, gslot, dstloc] = 1.0
        SexpT = np.zeros((128, max(NSLOT, 1), 128), np.float32)
        SexpT[dstloc, gslot, lane] = 1.0

        core_data.append(dict(
            idx=[np.tile(a, (8, 1)) for a in idx_a],
            Sagg=Sagg.astype(FP8), SexpT=SexpT.astype(FP8),
        ))

    sched = dict(slots=slots, batches=batches, NSLOT=NSLOT, idx_off=idx_off,
                 idx_cols=[max(c, 1) for c in idx_cols], core_data=core_data,
                 slot_tile=slot_tile, slot_base=slot_base)
    return sched


def build_program(cfg, sched, debug=False):
    from concourse import bacc, tile, mybir
    MAXPH = int(os.environ.get("GAT_MAX_PHASE", "99"))

    dt = mybir.dt
    nc = bacc.Bacc("TRN2", target_bir_lowering=False, debug=debug,
                   num_devices=cfg.NCORES)

    H, C, HC = cfg.H, cfg.C, cfg.HC
    NT, NSLOT = cfg.NT, sched['NSLOT']
    NB = cfg.NBATCH
    D1_GRP = 2
    N_ALL_T = cfg.N_ALL_PAD // 128
    WCOL = HC + 2 * H

    # ---------------- inputs ----------------
    xT_d = nc.dram_tensor("xT", [cfg.F_IN, cfg.N_ALL_PAD], dt.bfloat16, kind="ExternalInput")
    ald1_d = nc.dram_tensor("ald1", [128, NT, H], dt.bfloat16, kind="ExternalInput")
    W1e_d = nc.dram_tensor("W1e", [cfg.F_IN, HC + H], dt.bfloat16, kind="ExternalInput")
    W2e_d = nc.dram_tensor("W2e", [128, 2, WCOL], dt.bfloat16, kind="ExternalInput")
    W3e_d = nc.dram_tensor("W3e", [128, 2, WCOL], dt.bfloat16, kind="ExternalInput")
    b1_d = nc.dram_tensor("b1r", [128, HC], dt.float32, kind="ExternalInput")
    b2_d = nc.dram_tensor("b2r", [128, HC], dt.float32, kind="ExternalInput")
    b3_d = nc.dram_tensor("b3r", [128, C], dt.float32, kind="ExternalInput")
    idx_d = [nc.dram_tensor(f"idx{g}", [128, sched['idx_cols'][g]], dt.int16, kind="ExternalInput") for g in (0, 1)]
    Sagg_d = nc.dram_tensor("Sagg", [128, max(NSLOT, 1), 128], dt.float8e4, kind="ExternalInput")
    SexpT_d = nc.dram_tensor("SexpT", [128, max(NSLOT, 1), 128], dt.float8e4, kind="ExternalInput")
    Spool_d = nc.dram_tensor("Spool", [128, NT, 128], dt.float8e4, kind="ExternalInput")
    invc_d = nc.dram_tensor("invcnt", [128, 1], dt.float32, kind="ExternalInput")
    ident_d = nc.dram_tensor("ident", [128, 128], dt.float32, kind="ExternalInput")
    Wl_d = nc.dram_tensor("Wl", [C, cfg.NCOUT], dt.float32, kind="ExternalInput")
    out_d = nc.dram_tensor("out_part", [128, cfg.NCOUT], dt.float32, kind="ExternalOutput")

    # ---------------- internal DRAM ----------------
    table1_d = nc.dram_tensor("table1", [cfg.N_ALL_PAD, ROW], dt.bfloat16)
    X_d = [nc.dram_tensor(f"X{l}", [cfg.NODE_PAD, HC], dt.bfloat16) for l in (1, 2)]
    AGin_d = [nc.dram_tensor(f"AGin{l}", [cfg.NPC, ROW], dt.bfloat16) for l in (2, 3)]
    tabM_d = [nc.dram_tensor(f"tabM{l}", [cfg.N, ROW], dt.bfloat16, addr_space="Shared") for l in (2, 3)]

    batches = sched['batches']
    slots = sched['slots']
    idx_off = sched['idx_off']
    slot_tile = sched['slot_tile']

    def edge_phase(l, views, ald_sb, bias_t, pools, X_out, X3_sb):
        """Edge aggregation for layer l (1-based). views = (g0, g1) DRAM APs of
        the gather table (row stride ROW)."""
        ESUB = int(os.environ.get("GAT_E_SUB", "99"))
        mm = nc.tensor.matmul
        for b in range(NB):
            binfo = batches[b]
            NS_b = binfo['NS']
            if NS_b == 0:
                continue
            first = binfo['first']
            G = pools['G'].tile([128, NS_b, ROW], dt.bfloat16, tag="G")
            for g in (0, 1):
                gfirst, gn = binfo['g'][g]
                if gn == 0:
                    continue
                L = 8 * gn
                co = idx_off[(b, g)]
                it = pools['idx'].tile([128, L], dt.int16, tag="idx")
                nc.sync.dma_start(it[:], idx_d[g][:, co:co + L])
                o = gfirst - first
                # split into <=MAX_GATHER_SLOTS-slot gathers (descriptor ring cap)
                for s0 in range(0, gn, MAX_GATHER_SLOTS):
                    sn = min(MAX_GATHER_SLOTS, gn - s0)
                    nidx = 128 * sn
                    nc.gpsimd.dma_gather(
                        G[:, o + s0:o + s0 + sn, :], views[g],
                        it[:, 8 * s0:8 * (s0 + sn)], nidx, nidx, ROW)
            if ESUB < 2:
                continue
            S_t = pools['S'].tile([128, NS_b, 128], dt.float8e4, tag="S")
            ST_t = pools['S'].tile([128, NS_b, 128], dt.float8e4, tag="ST")
            nc.sync.dma_start(S_t[:], Sagg_d[:, first:first + NS_b, :])
            nc.sync.dma_start(ST_t[:], SexpT_d[:, first:first + NS_b, :])

            if ESUB < 3:
                continue
            # expansion: per-edge al_dst
            EX = pools['psE'].tile([128, NS_b, H], dt.float32, tag="EX")
            for s in range(NS_b):
                t = slot_tile[first + s]
                mm(EX[:, s, :], ST_t[:, s, :], ald_sb[:, t, :])
            if ESUB < 4:
                continue
            # logits -> exp
            e_f = pools['sm'].tile([128, NS_b, H], dt.float32, tag="e_f")
            nc.vector.tensor_tensor(e_f[:], G[:, :, HC:HC + H], EX[:], mybir.AluOpType.add)
            e_s = pools['sm'].tile([128, NS_b, H], dt.float32, tag="e_s")
            nc.vector.tensor_scalar_mul(e_s[:], e_f[:], cfg.neg)
            nc.vector.tensor_tensor(e_f[:], e_f[:], e_s[:], mybir.AluOpType.max)
            expw = pools['sm'].tile([128, NS_b, H], dt.float32, tag="expw")
            nc.scalar.activation(expw[:], e_f[:], mybir.ActivationFunctionType.Exp)
            expb = pools['sm'].tile([128, NS_b, H], dt.bfloat16, tag="expb")
            nc.vector.tensor_copy(expb[:], expw[:])
            if ESUB < 5:
                continue
            # messages in place: G[:, :, 0:HC] *= exp (per head)
            G4 = G[:, :, 0:HC].rearrange("p s (h f) -> p s h f", h=H)
            nc.vector.tensor_tensor(G4, G4, expb[:].unsqueeze(3).to_broadcast([128, NS_b, H, C]),
                                    mybir.AluOpType.mult)
            if ESUB < 6:
                continue
            # aggregation per tile
            for t in binfo['tiles']:
                tslots = []
                for g in (0, 1):
                    sb_ = int(sched['slot_base'][t, g])
                    tslots += list(range(sb_ - first, sb_ - first + int(slots[t, g])))
                if not tslots:
                    continue
                AGG = pools['psA'].tile([128, HC + H], dt.float32, tag="AGG")
                n = len(tslots)
                for i, s in enumerate(tslots):
                    mm(AGG[:, 0:HC], S_t[:, s, :], G[:, s, 0:HC], start=(i == 0), stop=(i == n - 1))
                for i, s in enumerate(tslots):
                    mm(AGG[:, HC:HC + H], S_t[:, s, :], expb[:, s, :], start=(i == 0), stop=(i == n - 1))
                if ESUB < 7:
                    continue
                # epilogue
                zc = pools['sm'].tile([128, H], dt.float32, tag="zc")
                nc.vector.tensor_scalar_max(zc[:], AGG[:, HC:HC + H], 1e-30)
                rz = pools['sm'].tile([128, H], dt.float32, tag="rz")
                nc.vector.reciprocal(rz[:], zc[:])
                if l < 3:
                    tmp = pools['ep'].tile([128, HC], dt.float32, tag="ep_tmp")
                    nc.vector.tensor_tensor(
                        tmp[:].rearrange("p (h f) -> p h f", h=H),
                        AGG[:, 0:HC].rearrange("p (h f) -> p h f", h=H),
                        rz[:].unsqueeze(2).to_broadcast([128, H, C]),
                        mybir.AluOpType.mult)
                    nc.vector.tensor_tensor(tmp[:], tmp[:], bias_t[:], mybir.AluOpType.add)
                    Xt = pools['ep'].tile([128, HC], dt.bfloat16, tag="ep_X")
                    nc.vector.tensor_scalar_max(Xt[:], tmp[:], 0.0)
                    nc.sync.dma_start(X_out[128 * t:128 * t + 128, :], Xt[:])
                else:
                    nc.vector.tensor_scalar_mul(rz[:], rz[:], 1.0 / H)
                    tmp = pools['ep'].tile([128, HC], dt.float32, tag="ep_tmp")
                    nc.vector.tensor_tensor(
                        tmp[:].rearrange("p (h f) -> p h f", h=H),
                        AGG[:, 0:HC].rearrange("p (h f) -> p h f", h=H),
                        rz[:].unsqueeze(2).to_broadcast([128, H, C]),
                        mybir.AluOpType.mult)
                    t4 = tmp[:].rearrange("p (h f) -> p h f", h=H)
                    a01 = pools['ep'].tile([128, C], dt.float32, tag="ep_a01")
                    nc.vector.tensor_tensor(a01[:], t4[:, 0, :], t4[:, 1, :], mybir.AluOpType.add)
                    a23 = pools['ep'].tile([128, C], dt.float32, tag="ep_a23")
                    nc.vector.tensor_tensor(a23[:], t4[:, 2, :], t4[:, 3, :], mybir.AluOpType.add)
                    nc.vector.tensor_tensor(a01[:], a01[:], a23[:], mybir.AluOpType.add)
                    nc.vector.tensor_tensor(a01[:], a01[:], bias_t[:], mybir.AluOpType.add)
                    nc.vector.tensor_scalar_max(X3_sb[:, t, :], a01[:], 0.0)

    with tile.TileContext(nc) as tc:
        # ======== Phase D1: replicated dense x @ [W1|w_als1] -> table1 ========
        if MAXPH >= 1:
            with (
                tc.tile_pool(name="d1sb", bufs=1) as d1sb,
                tc.tile_pool(name="d1st", bufs=3) as d1st,
                tc.tile_pool(name="d1ps", bufs=3, space="PSUM") as d1ps,
            ):
                xT_sb = d1sb.tile([cfg.F_IN, cfg.N_ALL_PAD], dt.bfloat16)
                nc.sync.dma_start(xT_sb[:], xT_d[:])
                W1_sb = d1sb.tile([cfg.F_IN, HC + H], dt.bfloat16)
                nc.sync.dma_start(W1_sb[:], W1e_d[:])
                ngrp = (N_ALL_T + D1_GRP - 1) // D1_GRP
                for gi in range(ngrp):
                    t0 = gi * D1_GRP
                    nt = min(D1_GRP, N_ALL_T - t0)
                    ps = d1ps.tile([128, D1_GRP, 512], dt.float32, tag="d1ps")
                    for i in range(nt):
                        nc.tensor.matmul(ps[:, i, 0:HC + H], xT_sb[:, 128 * (t0 + i):128 * (t0 + i + 1)], W1_sb[:])
                    st = d1st.tile([128, D1_GRP, HC + H], dt.bfloat16, tag="d1st")
                    nc.vector.tensor_copy(st[:, 0:nt, :], ps[:, 0:nt, 0:HC + H])
                    nc.sync.dma_start(
                        table1_d[128 * t0:128 * t0 + 128 * nt, 0:HC + H].rearrange(
                            "(i p) f -> p i f", p=128),
                        st[:, 0:nt, :])

        # persistent tiles across layers
        with (
            tc.tile_pool(name="pers", bufs=1) as pers,
        ):
            ald_sb = pers.tile([128, NT, H], dt.bfloat16)
            b_t = pers.tile([128, HC], dt.float32)
            X3_sb = pers.tile([128, NT, C], dt.bfloat16)

            # ======== Layer 1 edge phase ========
            if MAXPH >= 2:
                nc.sync.dma_start(ald_sb[:], ald1_d[:])
                nc.sync.dma_start(b_t[:], b1_d[:])
                with (
                    tc.tile_pool(name="G", bufs=2) as pG,
                    tc.tile_pool(name="S", bufs=2) as pS,
                    tc.tile_pool(name="idx", bufs=4) as pidx,
                    tc.tile_pool(name="sm", bufs=4) as psm,
                    tc.tile_pool(name="ep", bufs=4) as pep,
                    tc.tile_pool(name="psE", bufs=2, space="PSUM") as psE,
                    tc.tile_pool(name="psA", bufs=5, space="PSUM") as psA,
                ):
                    pools = dict(G=pG, S=pS, idx=pidx, sm=psm, ep=pep, psE=psE, psA=psA)
                    edge_phase(1,
                               (table1_d[0:cfg.SPLIT_ID, :], table1_d[cfg.SPLIT_ID:, :]),
                               ald_sb, b_t, pools, X_d[0], None)

            # ======== Layers 2,3: dense + AG + edge ========
            for li, l in enumerate((2, 3)):
                We_d = W2e_d if l == 2 else W3e_d
                Xin = X_d[l - 2]
                AGin = AGin_d[li]
                tabM = tabM_d[li]
                if MAXPH >= 3 + 3 * li:
                    with (
                        tc.tile_pool(name="dsb", bufs=1) as dsb,
                        tc.tile_pool(name="dst_", bufs=3) as dst_,
                        tc.tile_pool(name="dps", bufs=3, space="PSUM") as dps,
                    ):
                        XT_sb = dsb.tile([128, 2, cfg.NODE_PAD], dt.bfloat16)
                        for kc in range(2):
                            nc.sync.dma_start(XT_sb[:, kc, :], Xin[:, 128 * kc:128 * (kc + 1)], transpose=True)
                        We_sb = dsb.tile([128, 2, WCOL], dt.bfloat16)
                        nc.sync.dma_start(We_sb[:], We_d[:])
                        for t in range(NT):
                            rows = 128 if t < NT - 1 else cfg.TLAST
                            ps = dps.tile([128, WCOL], dt.float32, tag="dps")
                            for kc in range(2):
                                nc.tensor.matmul(ps[:], XT_sb[:, kc, 128 * t:128 * (t + 1)], We_sb[:, kc, :],
                                                 start=(kc == 0), stop=(kc == 1))
                            st = dst_.tile([128, HC + H], dt.bfloat16, tag="dstage")
                            nc.vector.tensor_copy(st[:], ps[:, 0:HC + H])
                            nc.sync.dma_start(AGin[128 * t:128 * t + rows, 0:HC + H], st[0:rows, :])
                            nc.scalar.activation(ald_sb[:, t, :], ps[:, HC + H:HC + 2 * H],
                                                 mybir.ActivationFunctionType.Copy)
                if MAXPH >= 4 + 3 * li:
                    nc.gpsimd.collective_compute(
                        "AllGather", mybir.AluOpType.bypass,
                        replica_groups=[list(range(cfg.NCORES))],
                        ins=[AGin[:]],
                        outs=[tabM[:]],
                    )
                if MAXPH >= 5 + 3 * li:
                    nc.sync.dma_start(b_t[:, 0:(HC if l < 3 else C)], (b2_d if l == 2 else b3_d)[:])
                    with (
                        tc.tile_pool(name="G", bufs=2) as pG,
                        tc.tile_pool(name="S", bufs=2) as pS,
                        tc.tile_pool(name="idx", bufs=4) as pidx,
                        tc.tile_pool(name="sm", bufs=4) as psm,
                        tc.tile_pool(name="ep", bufs=4) as pep,
                        tc.tile_pool(name="psE", bufs=2, space="PSUM") as psE,
                        tc.tile_pool(name="psA", bufs=5, space="PSUM") as psA,
                    ):
                        pools = dict(G=pG, S=pS, idx=pidx, sm=psm, ep=pep, psE=psE, psA=psA)
                        edge_phase(l,
                                   (tabM[0:cfg.SPLIT_ID, :], tabM[cfg.SPLIT_ID:, :]),
                                   ald_sb,
                                   b_t if l == 2 else b_t[:, 0:C],
                                   pools, X_d[1] if l == 2 else None,
                                   None if l == 2 else X3_sb)

            # ======== Pooling + final linear ========
            if MAXPH >= 9:
                with (
                    tc.tile_pool(name="po", bufs=1) as po,
                    tc.tile_pool(name="pops", bufs=1, space="PSUM") as pops,
                ):
                    Spool_sb = po.tile([128, NT, 128], dt.float8e4)
                    nc.sync.dma_start(Spool_sb[:], Spool_d[:])
                    PP = pops.tile([128, C], dt.float32)
                    for t in range(NT):
                        nc.tensor.matmul(PP[:], Spool_sb[:, t, :], X3_sb[:, t, :],
                                         start=(t == 0), stop=(t == NT - 1))
                    invc_sb = po.tile([128, 1], dt.float32)
                    nc.sync.dma_start(invc_sb[:], invc_d[:])
                    pooled = po.tile([128, C], dt.float32)
                    nc.vector.tensor_scalar(pooled[:], PP[:], invc_sb[:], None, mybir.AluOpType.mult)
                    ident_sb = po.tile([128, 128], dt.float32)
                    nc.sync.dma_start(ident_sb[:], ident_d[:])
                    pT = pops.tile([C, 128], dt.float32)
                    nc.tensor.transpose(pT[:], pooled[:], ident_sb[:])
                    pT_sb = po.tile([C, 128], dt.float32)
                    nc.vector.tensor_copy(pT_sb[:], pT[:])
                    Wl_sb = po.tile([C, cfg.NCOUT], dt.float32)
                    nc.sync.dma_start(Wl_sb[:], Wl_d[:])
                    lin = pops.tile([128, cfg.NCOUT], dt.float32)
                    nc.tensor.matmul(lin[:], pT_sb[:], Wl_sb[:])
                    lin_sb = po.tile([128, cfg.NCOUT], dt.float32)
                    nc.vector.tensor_copy(lin_sb[:], lin[:])
                    nc.sync.dma_start(out_d[:], lin_sb[:])

    nc.compile()
    return nc


def make_inputs(cfg, sched, x, edge_index, batch, W1, a_src1, a_dst1, b1,
                W2, a_src2, a_dst2, b2, W3, a_src3, a_dst3, b3, Wl, bl):
    """Build the per-core in_maps."""
    H, C, HC = cfg.H, cfg.C, cfg.HC

    def wfold(W, a):
        # w[f, h] = sum_c W[f, h*C + c] * a[h, c]
        return np.einsum('fhc,hc->fh', W.reshape(W.shape[0], H, C), a)

    xT = np.zeros((cfg.F_IN, cfg.N_ALL_PAD), np.float32)
    xT[:, :cfg.N] = x.T
    ald1_full = x @ wfold(W1, a_dst1)     # [N, H]
    W1e = np.concatenate([W1, wfold(W1, a_src1)], axis=1)  # [F_IN, HC+H]

    def wext(W, a_s, a_d):
        return np.concatenate([W, wfold(W, a_s), wfold(W, a_d)], axis=1)

    W2e = wext(W2, a_src2, a_dst2).reshape(2, 128, HC + 2 * H).transpose(1, 0, 2)
    W3e = wext(W3, a_src3, a_dst3).reshape(2, 128, HC + 2 * H).transpose(1, 0, 2)

    b1r = np.tile(b1[None, :], (128, 1)).astype(np.float32)
    b2r = np.tile(b2[None, :], (128, 1)).astype(np.float32)
    b3r = np.tile(b3[None, :], (128, 1)).astype(np.float32)

    cnt = np.bincount(batch, minlength=cfg.NGRAPH).astype(np.float32)

    in_maps = []
    gbases = []
    for k in range(cfg.NCORES):
        cd = sched['core_data'][k]
        ald1_sl = np.zeros((128, cfg.NT, H), np.float32)
        for t in range(cfg.NT):
            r0 = cfg.NPC * k + 128 * t
            rows = min(128, max(0, cfg.NPC * (k + 1) - r0))
            if rows > 0:
                ald1_sl[:rows, t, :] = ald1_full[r0:r0 + rows]
        gbase = int(batch[cfg.NPC * k])
        gbases.append(gbase)
        Spool = np.zeros((128, cfg.NT, 128), np.float32)
        bloc = batch[cfg.NPC * k: cfg.NPC * (k + 1)] - gbase
        assert bloc.max() < 128, f"core {k} spans {bloc.max()+1} graphs"
        rr = np.arange(cfg.NPC)
        Spool[rr & 127, rr >> 7, bloc] = 1.0
        invc = np.zeros((128, 1), np.float32)
        ng = min(128, cfg.NGRAPH - gbase)
        invc[:ng, 0] = 1.0 / np.maximum(cnt[gbase:gbase + ng], 1.0)

        in_maps.append({
            "xT": xT.astype(BF16),
            "ald1": ald1_sl.astype(BF16),
            "W1e": W1e.astype(BF16),
            "W2e": W2e.astype(BF16),
            "W3e": W3e.astype(BF16),
            "b1r": b1r, "b2r": b2r, "b3r": b3r,
            "idx0": cd['idx'][0], "idx1": cd['idx'][1],
            "Sagg": cd['Sagg'], "SexpT": cd['SexpT'],
            "Spool": Spool.astype(FP8),
            "invcnt": invc,
            "ident": np.eye(128, dtype=np.float32),
            "Wl": Wl.astype(np.float32),
        })
    return in_maps, gbases


def assemble_output(cfg, parts, gbases, bl):
    acc = np.zeros((cfg.NGRAPH, cfg.NCOUT), np.float32)
    for k in range(cfg.NCORES):
        g0 = gbases[k]
        rows = min(128, cfg.NGRAPH - g0)
        acc[g0:g0 + rows] += parts[k][:rows]
    return acc + bl[None, :]


def _run_on_hw(nc, in_maps, ncores, trace=True):
    """Execute via PJRT on the 8 axon-tunneled cores; capture an NTFF profile
    for the hardware exec time (the image lacks antenv.axon_hooks, so drive
    the NTFF capture directly via ctypes like trn_boot does)."""
    import tempfile
    import glob as _glob
    from concourse import bass2jax

    hook = None
    if trace:
        try:
            if '/root/.axon_site' not in sys.path:
                sys.path.insert(0, '/root/.axon_site')
            from trn_agent_boot.trn_boot import _ntff_profile_via_ctypes
            hook = _ntff_profile_via_ctypes('/opt/axon/libaxon_pjrt.so')
        except Exception as e:
            print(f"ntff hook unavailable ({e}); running without profile")
            hook = None

    if hook is None:
        results = bass2jax.run_bass_via_pjrt(nc, in_maps, n_cores=ncores)
        return results, None

    tmpdir = tempfile.mkdtemp(prefix="gat_ntff_")
    try:
        with hook(tmpdir, None):
            results = bass2jax.run_bass_via_pjrt(nc, in_maps, n_cores=ncores)
    except Exception as e:
        print(f"profiled run failed ({e}); retrying without profile")
        results = bass2jax.run_bass_via_pjrt(nc, in_maps, n_cores=ncores)
        return results, None

    exec_ns = None
    try:
        ntffs = _glob.glob(os.path.join(tmpdir, "*.ntff"))
        if not ntffs:
            print(f"no NTFF produced in {tmpdir}: {os.listdir(tmpdir)}")
            return results, None
        import gauge.profiler
        from concourse.bass_utils import _process_ntff_profile
        from concourse._compat import FishPath
        profile = gauge.profiler.Profile(
            profile_path=FishPath(tmpdir),
            kernel_dev_mode=True,
            profile_on_exit=False,
            bass_kernel=nc.m,
            offline_processing=True,
            fname="*_body*",
        )
        pr = _process_ntff_profile(profile, tmpdir, nc, list(range(ncores)),
                                   None, False, {}, False)
        exec_ns = pr.exec_time_ns
        print(f"profile dir: {tmpdir}")
    except Exception as e:
        import traceback
        traceback.print_exc()
        print(f"profile processing failed: {e}")
    return results, exec_ns


def run(cfg, inputs, use_sim=False, trace=True):
    global LAST_HW_NS
    x = np.asarray(inputs['x'], np.float32)
    ei = np.asarray(inputs['edge_index'])
    bt = np.asarray(inputs['batch']).astype(np.int64)
    loops = np.arange(cfg.N, dtype=np.int64)
    src = np.concatenate([ei[0].astype(np.int64), loops])
    dst = np.concatenate([ei[1].astype(np.int64), loops])

    sched = build_schedule(cfg, src, dst)
    nc = build_program(cfg, sched, debug=False)
    dense = {k: np.asarray(inputs[k], np.float32) for k in
             ('W1', 'a_src1', 'a_dst1', 'b1', 'W2', 'a_src2', 'a_dst2', 'b2',
              'W3', 'a_src3', 'a_dst3', 'b3', 'Wl', 'bl')}
    in_maps, gbases = make_inputs(
        cfg, sched, x, ei, bt,
        dense['W1'], dense['a_src1'], dense['a_dst1'], dense['b1'],
        dense['W2'], dense['a_src2'], dense['a_dst2'], dense['b2'],
        dense['W3'], dense['a_src3'], dense['a_dst3'], dense['b3'],
        dense['Wl'], dense['bl'])

    if use_sim:
        from concourse.bass_interp import MultiCoreSim
        sim = MultiCoreSim(nc, num_cores=cfg.NCORES, require_finite=False, require_nnan=False)
        for k in range(cfg.NCORES):
            for name, val in in_maps[k].items():
                sim.cores[k].tensor(name)[:] = val
        sim.simulate(check_with_hw=False)
        parts = [np.asarray(sim.cores[k].tensor("out_part")) for k in range(cfg.NCORES)]
    else:
        results, exec_ns = _run_on_hw(nc, in_maps, cfg.NCORES, trace=trace)
        if exec_ns:
            LAST_HW_NS = int(exec_ns)
        parts = [results[k]["out_part"] for k in range(cfg.NCORES)]

    return assemble_output(cfg, parts, gbases, dense['bl'])


def kernel(x, edge_index, batch, W1, a_src1, a_dst1, b1, W2, a_src2, a_dst2, b2,
           W3, a_src3, a_dst3, b3, Wl, bl):
    cfg = Cfg()
    return run(cfg, dict(x=x, edge_index=edge_index, batch=batch, W1=W1,
                         a_src1=a_src1, a_dst1=a_dst1, b1=b1, W2=W2,
                         a_src2=a_src2, a_dst2=a_dst2, b2=b2, W3=W3,
                         a_src3=a_src3, a_dst3=a_dst3, b3=b3, Wl=Wl, bl=bl))
